# revision 5
# baseline (speedup 1.0000x reference)
"""Trainium2 Bass kernel for nn_AttentionSubModule (B=262144, Q=25, D=9).

Strategy (pure data parallel over 8 NeuronCores, 32768 elements/core):
  - batch-on-partitions layout, chunks of 128 elements
  - PE: transpose x -> fullT, static block-diag projection matmuls (K, V with
    bias folded via a ones-row), transpose K/V back to batch layout
  - ACT: all PSUM->SBUF copies, exp (f32, no max subtraction -- scores are
    bounded ~[-20,35], f32 exp is safe), rsqrt via Ln+Exp (keeps every
    activation in the single natural_log_exp_and_others table set: no
    ACT table reloads inside the loop)
  - DVE: scores = K K^T (bcast multiply + reduce), softmax normalize into
    A = e/rowsum (f16), attn @ V, residual + LayerNorm
Dtypes: fp32 I/O and accumulations; fp16 internal operands.
"""

import numpy as np

import bass_rust as br
import concourse.bass as bass
import concourse.mybir as mybir
import concourse.tile as tile
from concourse.bass_utils import run_bass_kernel_spmd
from concourse.vector_clock import ScopedClock

B, Q, D = 262144, 25, 9
SEGS = [(0, 3), (3, 13), (13, 23), (23, 25)]
EPS = 1e-5
N_CORES = 8
BC = B // N_CORES          # elements per core
CH = 128                   # elements per compute chunk
SUP = 8                    # chunks per DMA super-chunk
DP = 10                    # padded d size (even, for 16-bit 2x mode)
QP = 26                    # padded q' size
KF = Q * DP                # 250   K fullT rows / K_b free size
VF = D * QP                # 234   V fullT rows / V_b free size

F32 = mybir.dt.float32
F16 = mybir.dt.float16
AX = mybir.AxisListType
OP = mybir.AluOpType
ACTF = mybir.ActivationFunctionType


def _split_multi_waits(nc, max_waits=1):
    """walrus here rejects instructions with more than one sync-wait command.
    Hoist extra waits onto same-engine NOPs inserted just before the
    offending instruction (same-engine program order makes this equivalent)."""
    for bb in nc.main_func.blocks:
        insts = bb.instructions
        out = []
        changed = False
        for inst in insts:
            si = getattr(inst, "sync_info", None)
            if si is not None and len(si.on_wait) > max_waits:
                waits = list(si.on_wait)
                keep = waits[: max_waits]
                extra = waits[max_waits:]
                for w in extra:
                    nop = mybir.InstNoOp(
                        name=f"wsplit_{nc.next_id()}", ins=[], outs=[]
                    )
                    nop.engine = inst.engine
                    nop.sync_info = br.SyncInfo(on_wait=[w], on_update=[])
                    out.append(nop)
                inst.sync_info = br.SyncInfo(
                    on_wait=keep, on_update=list(si.on_update)
                )
                changed = True
            out.append(inst)
        if changed:
            bb.instructions = out


def _patch_tile_drain():
    """walrus here rejects >1 sync-wait on the Tile tail Drain; spread the
    waits over single-wait NOPs instead."""

    def _drain_and_barrier(self, tick_clock, wait_clock):
        nc = self.nc
        probe = nc.sync.nop(nofuse=True)
        wait_clock.add_sem_waits(
            probe.ins, ScopedClock({None: tick_clock.global_clock})
        )
        si = probe.ins.sync_info
        if si is not None and len(si.on_wait) > 1:
            waits = list(si.on_wait)
            probe.ins.sync_info = br.SyncInfo(
                on_wait=[waits[0]], on_update=list(si.on_update)
            )
            for w in waits[1:]:
                n = nc.sync.nop(nofuse=True)
                n.ins.sync_info = br.SyncInfo(on_wait=[w], on_update=[])
        nc.sync.drain()

        nc.all_engine_barrier()
        assert self.sems is not None
        popped = nc._tile_sem_poison_stack.pop()
        assert popped is self._sem_poison
        nc.clear_and_free_semaphores(list(self.sems.allocated().values()))
        nc.all_engine_barrier()

    tile.TileContext._drain_and_barrier = _drain_and_barrier


_patch_tile_drain()


def _seg_of(q):
    for si, (s, e) in enumerate(SEGS):
        if s <= q < e:
            return si
    raise ValueError(q)


def make_weights(inp):
    """Host-side packing of the static stationary matrices.

    WK [226, 250]: K-proj.  out column m=(q*10+d) [d<9], contraction row
      k=(qt*9+dp) for qt<25 plus bias row k=225.
      WK[qt*9+dp, q*10+d] = Wk_seg(q)[d, dp] * (qt==q);  WK[225, q*10+d] = bk[d]
    WV [226, 234]: V-proj in (d, q')-major output order, m=(d*26+q') [q'<25].
      WV[qt*9+dp, d*26+qp] = Wv_seg(qp)[d, dp] * (qt==qp); WV[225, ...] = bv[d]
    """
    Wk = [np.asarray(inp[n], np.float32) for n in ("W_jk", "W_ok", "W_gk", "W_bk")]
    bk = [np.asarray(inp[n], np.float32) for n in ("b_jk", "b_ok", "b_gk", "b_bk")]
    Wv = [np.asarray(inp[n], np.float32) for n in ("W_jv", "W_ov", "W_gv", "W_bv")]
    bv = [np.asarray(inp[n], np.float32) for n in ("b_jv", "b_ov", "b_gv", "b_bv")]

    WK = np.zeros((226, KF), np.float32)
    WV = np.zeros((226, VF), np.float32)
    for q in range(Q):
        s = _seg_of(q)
        for d in range(D):
            for dp in range(D):
                WK[q * D + dp, q * DP + d] = Wk[s][d, dp]
                WV[q * D + dp, d * QP + q] = Wv[s][d, dp]
            WK[225, q * DP + d] = bk[s][d]
            WV[225, d * QP + q] = bv[s][d]
    return WK, WV


def build_nc(n_super, ln_trivial=True):
    """Build the single-core program processing n_super*SUP*CH elements."""
    n_el = n_super * SUP * CH
    nc = bass.Bass("TRN2", target_bir_lowering=False, debug=False)

    x_d = nc.dram_tensor("x", [n_el, Q * D], F32, kind="ExternalInput")
    y_d = nc.dram_tensor("y", [n_el, Q * D], F32, kind="ExternalOutput")
    wk_d = nc.dram_tensor("wk", [226, KF], F16, kind="ExternalInput")
    wv_d = nc.dram_tensor("wv", [226, VF], F16, kind="ExternalInput")
    id_d = nc.dram_tensor("ident", [128, 128], F16, kind="ExternalInput")
    idf_d = nc.dram_tensor("identf", [128, 128], F32, kind="ExternalInput")
    g_d = nc.dram_tensor("ln_g", [D], F32, kind="ExternalInput")
    b_d = nc.dram_tensor("ln_b", [D], F32, kind="ExternalInput")

    with tile.TileContext(nc) as tc:
        with (
            tc.tile_pool(name="singles", bufs=1) as singles,
            tc.tile_pool(name="xio", bufs=2) as xio,
            tc.tile_pool(name="yio", bufs=2) as yio,
            tc.tile_pool(name="kv", bufs=2) as kv,
            tc.tile_pool(name="big", bufs=2) as big,
            tc.tile_pool(name="small", bufs=3) as small,
            tc.tile_pool(name="ps", bufs=2, space="PSUM") as ps,
            tc.tile_pool(name="ps2", bufs=2, space="PSUM") as ps2,
        ):
            # --- static tiles -------------------------------------------------
            wka = singles.tile([128, KF], F16, tag="wka")
            wkb = singles.tile([98, KF], F16, tag="wkb")
            wva = singles.tile([128, VF], F16, tag="wva")
            wvb = singles.tile([98, VF], F16, tag="wvb")
            nc.sync.dma_start(out=wka, in_=wk_d[0:128, :])
            nc.sync.dma_start(out=wkb[0:98, :], in_=wk_d[128:226, :])
            nc.sync.dma_start(out=wva, in_=wv_d[0:128, :])
            nc.sync.dma_start(out=wvb[0:98, :], in_=wv_d[128:226, :])
            ident = singles.tile([128, 128], F16, tag="ident")
            identf = singles.tile([128, 128], F32, tag="identf")
            nc.sync.dma_start(out=ident, in_=id_d[:, :])
            nc.sync.dma_start(out=identf, in_=idf_d[:, :])
            eps_t = singles.tile([128, 1], F32, tag="eps")
            nc.vector.memset(eps_t, EPS)
            if not ln_trivial:
                g_rep = singles.tile([128, D], F32, tag="g_rep")
                b_rep = singles.tile([128, D], F32, tag="b_rep")
                nc.gpsimd.dma_start(out=g_rep, in_=g_d.ap().partition_broadcast(128))
                nc.gpsimd.dma_start(out=b_rep, in_=b_d.ap().partition_broadcast(128))

            x_sup_v = x_d.ap().rearrange("(s j p) f -> s p j f", p=CH, j=SUP)
            y_sup_v = y_d.ap().rearrange("(s j p) f -> s p j f", p=CH, j=SUP)

            for s in range(n_super):
                x_sup = xio.tile([CH, SUP, Q * D], F32, tag="x_sup")
                nc.sync.dma_start(out=x_sup, in_=x_sup_v[s])
                y_sup = yio.tile([CH, SUP, Q * D], F32, tag="y_sup")

                for j in range(SUP):
                    x32 = x_sup[:, j, :]  # [128, 225] fp32

                    # ---- transpose x to fullT ------------------------------
                    psx = ps.tile([128, 256], F32, tag="psx")
                    pxa = psx[:, 0:128]
                    pxb = psx[0:97, 128:256]
                    nc.tensor.transpose(pxa, x32[:, 0:128], identf)
                    nc.tensor.transpose(pxb, x32[:, 128:225], identf)
                    xta = kv.tile([128, 128], F16, tag="xta")
                    xtb = kv.tile([98, 128], F16, tag="xtb")
                    nc.scalar.copy(out=xta, in_=pxa)
                    nc.gpsimd.memset(xtb, 1.0)
                    nc.scalar.copy(out=xtb[0:97, :], in_=pxb)

                    # ---- projections (K fullT, V fullT-dT) -----------------
                    pskv = ps.tile([128, 512], F32, tag="pskv")
                    pka = pskv[:, 0:128]
                    pkb = pskv[0:122, 128:256]
                    pva = pskv[:, 256:384]
                    pvb = pskv[0:106, 384:512]
                    nc.tensor.matmul(pka, wka[:, 0:128], xta, start=True, stop=False)
                    nc.tensor.matmul(pka, wkb[:, 0:128], xtb, start=False, stop=True)
                    nc.tensor.matmul(pkb, wka[:, 128:KF], xta, start=True, stop=False)
                    nc.tensor.matmul(pkb, wkb[:, 128:KF], xtb, start=False, stop=True)
                    nc.tensor.matmul(pva, wva[:, 0:128], xta, start=True, stop=False)
                    nc.tensor.matmul(pva, wvb[:, 0:128], xtb, start=False, stop=True)
                    nc.tensor.matmul(pvb, wva[:, 128:VF], xta, start=True, stop=False)
                    nc.tensor.matmul(pvb, wvb[:, 128:VF], xtb, start=False, stop=True)

                    kta = kv.tile([128, 128], F16, tag="kta")
                    ktb = kv.tile([122, 128], F16, tag="ktb")
                    vta = kv.tile([128, 128], F16, tag="vta")
                    vtb = kv.tile([106, 128], F16, tag="vtb")
                    nc.scalar.copy(out=kta, in_=pka)
                    nc.scalar.copy(out=ktb, in_=pkb)
                    nc.scalar.copy(out=vta, in_=pva)
                    nc.scalar.copy(out=vtb, in_=pvb)

                    # ---- transpose K, V back to batch layout ---------------
                    pst = ps2.tile([128, 512], F16, tag="pst")
                    pkba = pst[:, 0:128]
                    pkbb = pst[:, 128:250]
                    pvba = pst[:, 256:384]
                    pvbb = pst[:, 384:490]
                    nc.tensor.transpose(pkba, kta, ident)
                    nc.tensor.transpose(pkbb, ktb, ident[0:122, 0:122])
                    nc.tensor.transpose(pvba, vta, ident)
                    nc.tensor.transpose(pvbb, vtb, ident[0:106, 0:106])

                    kb = kv.tile([128, KF], F16, tag="kb")
                    vb = kv.tile([128, VF], F16, tag="vb")
                    nc.scalar.copy(out=kb[:, 0:128], in_=pkba)
                    nc.scalar.copy(out=kb[:, 128:KF], in_=pkbb)
                    nc.scalar.copy(out=vb[:, 0:128], in_=pvba)
                    nc.scalar.copy(out=vb[:, 128:VF], in_=pvbb)

                    # ---- scores = K K^T ------------------------------------
                    k3 = kb.rearrange("c (q d) -> c q d", d=DP)       # [128,25,10]
                    prod = big.tile([128, Q, Q, DP], F16, tag="prod")
                    nc.vector.tensor_tensor(
                        out=prod,
                        in0=k3.unsqueeze(2).broadcast_to((128, Q, Q, DP)),
                        in1=k3.unsqueeze(1).broadcast_to((128, Q, Q, DP)),
                        op=OP.mult,
                    )
                    scores = big.tile([128, Q, QP], F32, tag="scores")
                    nc.vector.tensor_reduce(
                        out=scores[:, :, 0:Q],
                        in_=prod[:, :, :, 0:D],
                        axis=AX.X,
                        op=OP.add,
                    )

                    # ---- softmax (no max subtraction; scores bounded) ------
                    e_t = big.tile([128, Q, QP], F32, tag="e_t")
                    nc.gpsimd.memset(e_t[:, :, Q:QP], 0.0)
                    nc.scalar.activation(
                        out=e_t[:, :, 0:Q],
                        in_=scores[:, :, 0:Q],
                        func=ACTF.Exp,
                    )
                    rsum = small.tile([128, Q], F32, tag="rsum")
                    nc.vector.tensor_reduce(
                        out=rsum, in_=e_t[:, :, 0:Q], axis=AX.X, op=OP.add
                    )
                    rinv = small.tile([128, Q], F32, tag="rinv")
                    nc.vector.reciprocal(out=rinv, in_=rsum)
                    a_t = big.tile([128, Q, QP], F16, tag="a_t")
                    nc.vector.tensor_tensor(
                        out=a_t,
                        in0=e_t,
                        in1=rinv.unsqueeze(2).broadcast_to((128, Q, QP)),
                        op=OP.mult,
                    )

                    # ---- res = A @ V ---------------------------------------
                    v3 = vb.rearrange("c (d qp) -> c d qp", qp=QP)    # [128,9,26]
                    prod2 = big.tile([128, Q, D, QP], F16, tag="prod2")
                    nc.vector.tensor_tensor(
                        out=prod2,
                        in0=a_t.unsqueeze(2).broadcast_to((128, Q, D, QP)),
                        in1=v3.unsqueeze(1).broadcast_to((128, Q, D, QP)),
                        op=OP.mult,
                    )
                    res = small.tile([128, Q, D], F32, tag="res")
                    nc.vector.tensor_reduce(
                        out=res, in_=prod2[:, :, :, 0:Q], axis=AX.X, op=OP.add
                    )

                    # ---- y = x + res; LayerNorm over d ---------------------
                    x3 = x32.rearrange("c (q d) -> c q d", d=D)
                    yt = small.tile([128, Q, D], F32, tag="yt")
                    nc.vector.tensor_tensor(out=yt, in0=res, in1=x3, op=OP.add)

                    msum = small.tile([128, Q], F32, tag="msum")
                    nc.vector.tensor_reduce(out=msum, in_=yt, axis=AX.X, op=OP.add)
                    negmean = small.tile([128, Q], F32, tag="negmean")
                    nc.vector.tensor_scalar_mul(negmean, msum, -1.0 / D)
                    yc = small.tile([128, Q, D], F32, tag="yc")
                    nc.vector.tensor_tensor(
                        out=yc,
                        in0=yt,
                        in1=negmean.unsqueeze(2).broadcast_to((128, Q, D)),
                        op=OP.add,
                    )
                    sq = small.tile([128, Q, D], F32, tag="sq")
                    nc.scalar.activation(out=sq, in_=yc, func=ACTF.Square)
                    vsum = small.tile([128, Q], F32, tag="vsum")
                    nc.vector.tensor_reduce(out=vsum, in_=sq, axis=AX.X, op=OP.add)
                    # 1/sqrt(var+eps) = exp(-0.5 * ln(var+eps)); Ln and Exp
                    # share one ACT table set (natural_log_exp_and_others)
                    lnv = small.tile([128, Q], F32, tag="lnv")
                    nc.scalar.activation(
                        out=lnv, in_=vsum, func=ACTF.Ln, bias=eps_t, scale=1.0 / D
                    )
                    sdinv = small.tile([128, Q], F32, tag="sdinv")
                    nc.scalar.activation(
                        out=sdinv, in_=lnv, func=ACTF.Exp, scale=-0.5
                    )

                    y3 = y_sup[:, j, :].rearrange("c (q d) -> c q d", d=D)
                    if ln_trivial:
                        nc.vector.tensor_tensor(
                            out=y3,
                            in0=yc,
                            in1=sdinv.unsqueeze(2).broadcast_to((128, Q, D)),
                            op=OP.mult,
                        )
                    else:
                        t2 = small.tile([128, Q, D], F32, tag="t2")
                        nc.vector.tensor_tensor(
                            out=t2,
                            in0=yc,
                            in1=sdinv.unsqueeze(2).broadcast_to((128, Q, D)),
                            op=OP.mult,
                        )
                        t3 = small.tile([128, Q, D], F32, tag="t3")
                        nc.vector.tensor_tensor(
                            out=t3,
                            in0=t2,
                            in1=g_rep.unsqueeze(1).broadcast_to((128, Q, D)),
                            op=OP.mult,
                        )
                        nc.vector.tensor_tensor(
                            out=y3,
                            in0=t3,
                            in1=b_rep.unsqueeze(1).broadcast_to((128, Q, D)),
                            op=OP.add,
                        )

                nc.sync.dma_start(out=y_sup_v[s], in_=y_sup)

    _split_multi_waits(nc)
    return nc


def _host_inputs(inputs):
    x = np.ascontiguousarray(np.asarray(inputs["x"], np.float32).reshape(-1, Q * D))
    WK, WV = make_weights(inputs)

    wk16 = WK.astype(np.float16)
    wv16 = WV.astype(np.float16)
    ident = np.eye(128, dtype=np.float16)
    identf = np.eye(128, dtype=np.float32)
    g = np.asarray(inputs["ln_g"], np.float32)
    b = np.asarray(inputs["ln_b"], np.float32)
    return x, wk16, wv16, ident, identf, g, b


def _run_and_bench(nc, in_maps, n_cores, bench_iters=8):
    """Mirror of bass2jax.run_bass_via_pjrt's multi-core path, without output
    donation so the jitted callable can be re-invoked on device-resident
    inputs to wall-time pure execution."""
    import time

    import jax
    import concourse.mybir as mb
    from concourse.bass2jax import (
        _bass_exec_p,
        install_neuronx_cc_hook,
        partition_id_tensor,
    )
    from jax.experimental.shard_map import shard_map
    from jax.sharding import Mesh, PartitionSpec

    install_neuronx_cc_hook()
    partition_name = nc.partition_id_tensor.name if nc.partition_id_tensor else None
    in_names, out_names, out_avals, zero_outs = [], [], [], []
    for alloc in nc.m.functions[0].allocations:
        if not isinstance(alloc, mb.MemoryLocationSet):
            continue
        name = alloc.memorylocations[0].name
        if alloc.kind == "ExternalInput":
            if name != partition_name:
                in_names.append(name)
        elif alloc.kind == "ExternalOutput":
            out_names.append(name)
            shape = tuple(alloc.tensor_shape)
            dtype = mb.dt.np(alloc.dtype)
            out_avals.append(jax.core.ShapedArray(shape, dtype))
            zero_outs.append(np.zeros(shape, dtype))
    n_params = len(in_names)
    all_in_names = in_names + out_names
    if partition_name is not None:
        all_in_names.append(partition_name)

    def _body(*args):
        operands = list(args)
        if partition_name is not None:
            operands.append(partition_id_tensor())
        outs = _bass_exec_p.bind(
            *operands,
            out_avals=tuple(out_avals),
            in_names=tuple(all_in_names),
            out_names=tuple(out_names),
            lowering_input_output_aliases=(),
            sim_require_finite=True,
            sim_require_nnan=True,
            nc=nc,
        )
        return tuple(outs)

    devices = jax.devices()[:n_cores]
    mesh = Mesh(np.asarray(devices), ("core",))
    nspec = n_params + len(out_avals)
    fn = jax.jit(
        shard_map(
            _body,
            mesh=mesh,
            in_specs=(PartitionSpec("core"),) * nspec,
            out_specs=(PartitionSpec("core"),) * len(out_names),
            check_rep=False,
        ),
        keep_unused=True,
    )
    concat_in = [
        np.concatenate([np.asarray(m[name]) for m in in_maps], axis=0)
        for name in in_names
    ]
    concat_zero = [
        np.zeros((n_cores * z.shape[0], *z.shape[1:]), z.dtype) for z in zero_outs
    ]
    dev_args = [jax.device_put(a) for a in concat_in + concat_zero]
    outs = fn(*dev_args)
    jax.block_until_ready(outs)

    exec_ns = None
    if bench_iters > 0:
        t0 = time.perf_counter()
        for _ in range(bench_iters):
            outs_b = fn(*dev_args)
        jax.block_until_ready(outs_b)
        exec_ns = (time.perf_counter() - t0) / bench_iters * 1e9

    results = []
    for c in range(n_cores):
        res = {}
        for i, name in enumerate(out_names):
            full = np.asarray(outs[i])
            per = full.shape[0] // n_cores
            res[name] = full[c * per : (c + 1) * per]
        results.append(res)
    return results, exec_ns


def kernel(**inputs):
    import os

    x, wk16, wv16, ident, identf, g, b = _host_inputs(inputs)
    n_el_total = x.shape[0]
    assert n_el_total % (N_CORES * SUP * CH) == 0
    bc = n_el_total // N_CORES
    n_super = bc // (SUP * CH)

    ln_trivial = bool(np.allclose(g, 1.0) and np.allclose(b, 0.0))
    nc = build_nc(n_super, ln_trivial=ln_trivial)
    in_maps = []
    for i in range(N_CORES):
        in_maps.append(
            {
                "x": x[i * bc : (i + 1) * bc],
                "wk": wk16,
                "wv": wv16,
                "ident": ident,
                "identf": identf,
                "ln_g": g,
                "ln_b": b,
            }
        )
    bench = int(os.environ.get("KERNEL_BENCH", "0"))
    global LAST_EXEC_TIME_NS
    if bench > 0:
        results, exec_ns = _run_and_bench(nc, in_maps, N_CORES, bench_iters=bench)
        LAST_EXEC_TIME_NS = exec_ns
        y = np.concatenate([results[i]["y"] for i in range(N_CORES)], axis=0)
    else:
        rr = run_bass_kernel_spmd(nc, in_maps, list(range(N_CORES)))
        LAST_EXEC_TIME_NS = rr.exec_time_ns
        y = np.concatenate([rr.results[i]["y"] for i in range(N_CORES)], axis=0)
    return y.reshape(np.asarray(inputs["x"]).shape)


LAST_EXEC_TIME_NS = None


# revision 6
# speedup vs baseline: 1.0132x; 1.0132x over previous
"""Trainium2 Bass kernel for nn_AttentionSubModule (B=262144, Q=25, D=9).

Strategy (pure data parallel over 8 NeuronCores, 32768 elements/core):
  - batch-on-partitions layout, chunks of 128 elements
  - PE: transpose x -> fullT, static block-diag projection matmuls (K, V with
    bias folded via a ones-row), transpose K/V back to batch layout
  - ACT: all PSUM->SBUF copies, exp (f32, no max subtraction -- scores are
    bounded ~[-20,35], f32 exp is safe), rsqrt via Ln+Exp (keeps every
    activation in the single natural_log_exp_and_others table set: no
    ACT table reloads inside the loop)
  - DVE: scores = K K^T (bcast multiply + reduce), softmax normalize into
    A = e/rowsum (f16), attn @ V, residual + LayerNorm
Dtypes: fp32 I/O and accumulations; fp16 internal operands.
"""

import numpy as np

import bass_rust as br
import concourse.bass as bass
import concourse.mybir as mybir
import concourse.tile as tile
from concourse.bass_utils import run_bass_kernel_spmd
from concourse.vector_clock import ScopedClock

B, Q, D = 262144, 25, 9
SEGS = [(0, 3), (3, 13), (13, 23), (23, 25)]
EPS = 1e-5
N_CORES = 8
BC = B // N_CORES          # elements per core
CH = 128                   # elements per compute chunk
SUP = 8                    # chunks per DMA super-chunk
DP = 10                    # padded d size (even, for 16-bit 2x mode)
QP = 26                    # padded q' size
KF = Q * DP                # 250   K fullT rows / K_b free size
VF = D * QP                # 234   V fullT rows / V_b free size

F32 = mybir.dt.float32
F16 = mybir.dt.float16
AX = mybir.AxisListType
OP = mybir.AluOpType
ACTF = mybir.ActivationFunctionType


def _split_multi_waits(nc, max_waits=1):
    """walrus here rejects instructions with more than one sync-wait command.
    Hoist extra waits onto same-engine NOPs inserted just before the
    offending instruction (same-engine program order makes this equivalent)."""
    for bb in nc.main_func.blocks:
        insts = bb.instructions
        out = []
        changed = False
        for inst in insts:
            si = getattr(inst, "sync_info", None)
            if si is not None and len(si.on_wait) > max_waits:
                waits = list(si.on_wait)
                keep = waits[: max_waits]
                extra = waits[max_waits:]
                for w in extra:
                    nop = mybir.InstNoOp(
                        name=f"wsplit_{nc.next_id()}", ins=[], outs=[]
                    )
                    nop.engine = inst.engine
                    nop.sync_info = br.SyncInfo(on_wait=[w], on_update=[])
                    out.append(nop)
                inst.sync_info = br.SyncInfo(
                    on_wait=keep, on_update=list(si.on_update)
                )
                changed = True
            out.append(inst)
        if changed:
            bb.instructions = out


def _patch_tile_drain():
    """walrus here rejects >1 sync-wait on the Tile tail Drain; spread the
    waits over single-wait NOPs instead."""

    def _drain_and_barrier(self, tick_clock, wait_clock):
        nc = self.nc
        probe = nc.sync.nop(nofuse=True)
        wait_clock.add_sem_waits(
            probe.ins, ScopedClock({None: tick_clock.global_clock})
        )
        si = probe.ins.sync_info
        if si is not None and len(si.on_wait) > 1:
            waits = list(si.on_wait)
            probe.ins.sync_info = br.SyncInfo(
                on_wait=[waits[0]], on_update=list(si.on_update)
            )
            for w in waits[1:]:
                n = nc.sync.nop(nofuse=True)
                n.ins.sync_info = br.SyncInfo(on_wait=[w], on_update=[])
        nc.sync.drain()

        nc.all_engine_barrier()
        assert self.sems is not None
        popped = nc._tile_sem_poison_stack.pop()
        assert popped is self._sem_poison
        nc.clear_and_free_semaphores(list(self.sems.allocated().values()))
        nc.all_engine_barrier()

    tile.TileContext._drain_and_barrier = _drain_and_barrier


_patch_tile_drain()


def _seg_of(q):
    for si, (s, e) in enumerate(SEGS):
        if s <= q < e:
            return si
    raise ValueError(q)


def make_weights(inp):
    """Host-side packing of the static stationary matrices.

    WK [226, 250]: K-proj.  out column m=(q*10+d) [d<9], contraction row
      k=(qt*9+dp) for qt<25 plus bias row k=225.
      WK[qt*9+dp, q*10+d] = Wk_seg(q)[d, dp] * (qt==q);  WK[225, q*10+d] = bk[d]
    WV [226, 234]: V-proj in (d, q')-major output order, m=(d*26+q') [q'<25].
      WV[qt*9+dp, d*26+qp] = Wv_seg(qp)[d, dp] * (qt==qp); WV[225, ...] = bv[d]
    """
    Wk = [np.asarray(inp[n], np.float32) for n in ("W_jk", "W_ok", "W_gk", "W_bk")]
    bk = [np.asarray(inp[n], np.float32) for n in ("b_jk", "b_ok", "b_gk", "b_bk")]
    Wv = [np.asarray(inp[n], np.float32) for n in ("W_jv", "W_ov", "W_gv", "W_bv")]
    bv = [np.asarray(inp[n], np.float32) for n in ("b_jv", "b_ov", "b_gv", "b_bv")]

    WK = np.zeros((226, KF), np.float32)
    WV = np.zeros((226, VF), np.float32)
    for q in range(Q):
        s = _seg_of(q)
        for d in range(D):
            for dp in range(D):
                WK[q * D + dp, q * DP + d] = Wk[s][d, dp]
                WV[q * D + dp, d * QP + q] = Wv[s][d, dp]
            WK[225, q * DP + d] = bk[s][d]
            WV[225, d * QP + q] = bv[s][d]
    return WK, WV


def build_nc(n_super, ln_trivial=True):
    """Build the single-core program processing n_super*SUP*CH elements."""
    n_el = n_super * SUP * CH
    nc = bass.Bass("TRN2", target_bir_lowering=False, debug=False)

    x_d = nc.dram_tensor("x", [n_el, Q * D], F32, kind="ExternalInput")
    y_d = nc.dram_tensor("y", [n_el, Q * D], F32, kind="ExternalOutput")
    wk_d = nc.dram_tensor("wk", [226, KF], F16, kind="ExternalInput")
    wv_d = nc.dram_tensor("wv", [226, VF], F16, kind="ExternalInput")
    id_d = nc.dram_tensor("ident", [128, 128], F16, kind="ExternalInput")
    idf_d = nc.dram_tensor("identf", [128, 128], F32, kind="ExternalInput")
    g_d = nc.dram_tensor("ln_g", [D], F32, kind="ExternalInput")
    b_d = nc.dram_tensor("ln_b", [D], F32, kind="ExternalInput")

    with tile.TileContext(nc) as tc:
        with (
            tc.tile_pool(name="singles", bufs=1) as singles,
            tc.tile_pool(name="xio", bufs=2) as xio,
            tc.tile_pool(name="yio", bufs=2) as yio,
            tc.tile_pool(name="kv", bufs=2) as kv,
            tc.tile_pool(name="big", bufs=2) as big,
            tc.tile_pool(name="small", bufs=3) as small,
            tc.tile_pool(name="ps", bufs=2, space="PSUM") as ps,
            tc.tile_pool(name="ps2", bufs=2, space="PSUM") as ps2,
        ):
            # --- static tiles -------------------------------------------------
            wka = singles.tile([128, KF], F16, tag="wka")
            wkb = singles.tile([98, KF], F16, tag="wkb")
            wva = singles.tile([128, VF], F16, tag="wva")
            wvb = singles.tile([98, VF], F16, tag="wvb")
            nc.sync.dma_start(out=wka, in_=wk_d[0:128, :])
            nc.sync.dma_start(out=wkb[0:98, :], in_=wk_d[128:226, :])
            nc.sync.dma_start(out=wva, in_=wv_d[0:128, :])
            nc.sync.dma_start(out=wvb[0:98, :], in_=wv_d[128:226, :])
            ident = singles.tile([128, 128], F16, tag="ident")
            identf = singles.tile([128, 128], F32, tag="identf")
            nc.sync.dma_start(out=ident, in_=id_d[:, :])
            nc.sync.dma_start(out=identf, in_=idf_d[:, :])
            eps_t = singles.tile([128, 1], F32, tag="eps")
            nc.vector.memset(eps_t, EPS)
            if not ln_trivial:
                g_rep = singles.tile([128, D], F32, tag="g_rep")
                b_rep = singles.tile([128, D], F32, tag="b_rep")
                nc.gpsimd.dma_start(out=g_rep, in_=g_d.ap().partition_broadcast(128))
                nc.gpsimd.dma_start(out=b_rep, in_=b_d.ap().partition_broadcast(128))

            x_sup_v = x_d.ap().rearrange("(s j p) f -> s p j f", p=CH, j=SUP)
            y_sup_v = y_d.ap().rearrange("(s j p) f -> s p j f", p=CH, j=SUP)

            for s in range(n_super):
                x_sup = xio.tile([CH, SUP, Q * D], F32, tag="x_sup")
                nc.sync.dma_start(out=x_sup, in_=x_sup_v[s])
                y_sup = yio.tile([CH, SUP, Q * D], F32, tag="y_sup")

                for j in range(SUP):
                    x32 = x_sup[:, j, :]  # [128, 225] fp32

                    # ---- transpose x to fullT ------------------------------
                    psx = ps.tile([128, 256], F32, tag="psx")
                    pxa = psx[:, 0:128]
                    pxb = psx[0:97, 128:256]
                    nc.tensor.transpose(pxa, x32[:, 0:128], identf)
                    nc.tensor.transpose(pxb, x32[:, 128:225], identf)
                    xta = kv.tile([128, 128], F16, tag="xta")
                    xtb = kv.tile([98, 128], F16, tag="xtb")
                    nc.scalar.copy(out=xta, in_=pxa)
                    nc.gpsimd.memset(xtb, 1.0)
                    nc.scalar.copy(out=xtb[0:97, :], in_=pxb)

                    # ---- projections (K fullT, V fullT-dT) -----------------
                    pskv = ps.tile([128, 512], F32, tag="pskv")
                    pka = pskv[:, 0:128]
                    pkb = pskv[0:122, 128:256]
                    pva = pskv[:, 256:384]
                    pvb = pskv[0:106, 384:512]
                    nc.tensor.matmul(pka, wka[:, 0:128], xta, start=True, stop=False)
                    nc.tensor.matmul(pka, wkb[:, 0:128], xtb, start=False, stop=True)
                    nc.tensor.matmul(pkb, wka[:, 128:KF], xta, start=True, stop=False)
                    nc.tensor.matmul(pkb, wkb[:, 128:KF], xtb, start=False, stop=True)
                    nc.tensor.matmul(pva, wva[:, 0:128], xta, start=True, stop=False)
                    nc.tensor.matmul(pva, wvb[:, 0:128], xtb, start=False, stop=True)
                    nc.tensor.matmul(pvb, wva[:, 128:VF], xta, start=True, stop=False)
                    nc.tensor.matmul(pvb, wvb[:, 128:VF], xtb, start=False, stop=True)

                    kta = kv.tile([128, 128], F16, tag="kta")
                    ktb = kv.tile([122, 128], F16, tag="ktb")
                    vta = kv.tile([128, 128], F16, tag="vta")
                    vtb = kv.tile([106, 128], F16, tag="vtb")
                    nc.scalar.copy(out=kta, in_=pka)
                    nc.scalar.copy(out=ktb, in_=pkb)
                    nc.scalar.copy(out=vta, in_=pva)
                    nc.scalar.copy(out=vtb, in_=pvb)

                    # ---- transpose K, V back to batch layout ---------------
                    pst = ps2.tile([128, 512], F16, tag="pst")
                    pkba = pst[:, 0:128]
                    pkbb = pst[:, 128:250]
                    pvba = pst[:, 256:384]
                    pvbb = pst[:, 384:490]
                    nc.tensor.transpose(pkba, kta, ident)
                    nc.tensor.transpose(pkbb, ktb, ident[0:122, 0:122])
                    nc.tensor.transpose(pvba, vta, ident)
                    nc.tensor.transpose(pvbb, vtb, ident[0:106, 0:106])

                    kb = kv.tile([128, KF], F16, tag="kb")
                    vb = kv.tile([128, VF], F16, tag="vb")
                    nc.scalar.copy(out=kb[:, 0:128], in_=pkba)
                    nc.scalar.copy(out=kb[:, 128:KF], in_=pkbb)
                    nc.scalar.copy(out=vb[:, 0:128], in_=pvba)
                    nc.scalar.copy(out=vb[:, 128:VF], in_=pvbb)

                    # ---- scores = K K^T ------------------------------------
                    k3 = kb.rearrange("c (q d) -> c q d", d=DP)       # [128,25,10]
                    prod = big.tile([128, Q, Q, DP], F16, tag="prod")
                    nc.vector.tensor_tensor(
                        out=prod,
                        in0=k3.unsqueeze(2).broadcast_to((128, Q, Q, DP)),
                        in1=k3.unsqueeze(1).broadcast_to((128, Q, Q, DP)),
                        op=OP.mult,
                    )
                    scores = big.tile([128, Q, QP], F32, tag="scores")
                    nc.vector.tensor_reduce(
                        out=scores[:, :, 0:Q],
                        in_=prod[:, :, :, 0:D],
                        axis=AX.X,
                        op=OP.add,
                    )

                    # ---- softmax (no max subtraction; scores bounded) ------
                    e_t = big.tile([128, Q, QP], F32, tag="e_t")
                    nc.gpsimd.memset(e_t[:, :, Q:QP], 0.0)
                    nc.scalar.activation(
                        out=e_t[:, :, 0:Q],
                        in_=scores[:, :, 0:Q],
                        func=ACTF.Exp,
                    )
                    rsum = small.tile([128, Q], F32, tag="rsum")
                    nc.vector.tensor_reduce(
                        out=rsum, in_=e_t[:, :, 0:Q], axis=AX.X, op=OP.add
                    )
                    rinv = small.tile([128, Q], F32, tag="rinv")
                    nc.vector.reciprocal(out=rinv, in_=rsum)
                    a_t = big.tile([128, Q, QP], F16, tag="a_t")
                    nc.gpsimd.tensor_tensor(
                        out=a_t,
                        in0=e_t,
                        in1=rinv.unsqueeze(2).broadcast_to((128, Q, QP)),
                        op=OP.mult,
                    )

                    # ---- res = A @ V ---------------------------------------
                    v3 = vb.rearrange("c (d qp) -> c d qp", qp=QP)    # [128,9,26]
                    prod2 = big.tile([128, Q, D, QP], F16, tag="prod2")
                    nc.vector.tensor_tensor(
                        out=prod2,
                        in0=a_t.unsqueeze(2).broadcast_to((128, Q, D, QP)),
                        in1=v3.unsqueeze(1).broadcast_to((128, Q, D, QP)),
                        op=OP.mult,
                    )
                    res = small.tile([128, Q, D], F32, tag="res")
                    nc.vector.tensor_reduce(
                        out=res, in_=prod2[:, :, :, 0:Q], axis=AX.X, op=OP.add
                    )

                    # ---- y = x + res; LayerNorm over d ---------------------
                    x3 = x32.rearrange("c (q d) -> c q d", d=D)
                    yt = small.tile([128, Q, D], F32, tag="yt")
                    nc.gpsimd.tensor_tensor(out=yt, in0=res, in1=x3, op=OP.add)

                    msum = small.tile([128, Q], F32, tag="msum")
                    nc.vector.tensor_reduce(out=msum, in_=yt, axis=AX.X, op=OP.add)
                    negmean = small.tile([128, Q], F32, tag="negmean")
                    nc.gpsimd.tensor_scalar_mul(negmean, msum, -1.0 / D)
                    yc = small.tile([128, Q, D], F32, tag="yc")
                    nc.gpsimd.tensor_tensor(
                        out=yc,
                        in0=yt,
                        in1=negmean.unsqueeze(2).broadcast_to((128, Q, D)),
                        op=OP.add,
                    )
                    sq = small.tile([128, Q, D], F32, tag="sq")
                    nc.scalar.activation(out=sq, in_=yc, func=ACTF.Square)
                    vsum = small.tile([128, Q], F32, tag="vsum")
                    nc.vector.tensor_reduce(out=vsum, in_=sq, axis=AX.X, op=OP.add)
                    # 1/sqrt(var+eps) = exp(-0.5 * ln(var+eps)); Ln and Exp
                    # share one ACT table set (natural_log_exp_and_others)
                    lnv = small.tile([128, Q], F32, tag="lnv")
                    nc.scalar.activation(
                        out=lnv, in_=vsum, func=ACTF.Ln, bias=eps_t, scale=1.0 / D
                    )
                    sdinv = small.tile([128, Q], F32, tag="sdinv")
                    nc.scalar.activation(
                        out=sdinv, in_=lnv, func=ACTF.Exp, scale=-0.5
                    )

                    y3 = y_sup[:, j, :].rearrange("c (q d) -> c q d", d=D)
                    if ln_trivial:
                        nc.vector.tensor_tensor(
                            out=y3,
                            in0=yc,
                            in1=sdinv.unsqueeze(2).broadcast_to((128, Q, D)),
                            op=OP.mult,
                        )
                    else:
                        t2 = small.tile([128, Q, D], F32, tag="t2")
                        nc.vector.tensor_tensor(
                            out=t2,
                            in0=yc,
                            in1=sdinv.unsqueeze(2).broadcast_to((128, Q, D)),
                            op=OP.mult,
                        )
                        t3 = small.tile([128, Q, D], F32, tag="t3")
                        nc.vector.tensor_tensor(
                            out=t3,
                            in0=t2,
                            in1=g_rep.unsqueeze(1).broadcast_to((128, Q, D)),
                            op=OP.mult,
                        )
                        nc.vector.tensor_tensor(
                            out=y3,
                            in0=t3,
                            in1=b_rep.unsqueeze(1).broadcast_to((128, Q, D)),
                            op=OP.add,
                        )

                nc.sync.dma_start(out=y_sup_v[s], in_=y_sup)

    _split_multi_waits(nc)
    return nc


def _host_inputs(inputs):
    x = np.ascontiguousarray(np.asarray(inputs["x"], np.float32).reshape(-1, Q * D))
    WK, WV = make_weights(inputs)

    wk16 = WK.astype(np.float16)
    wv16 = WV.astype(np.float16)
    ident = np.eye(128, dtype=np.float16)
    identf = np.eye(128, dtype=np.float32)
    g = np.asarray(inputs["ln_g"], np.float32)
    b = np.asarray(inputs["ln_b"], np.float32)
    return x, wk16, wv16, ident, identf, g, b


def _run_and_bench(nc, in_maps, n_cores, bench_iters=8):
    """Mirror of bass2jax.run_bass_via_pjrt's multi-core path, without output
    donation so the jitted callable can be re-invoked on device-resident
    inputs to wall-time pure execution."""
    import time

    import jax
    import concourse.mybir as mb
    from concourse.bass2jax import (
        _bass_exec_p,
        install_neuronx_cc_hook,
        partition_id_tensor,
    )
    from jax.experimental.shard_map import shard_map
    from jax.sharding import Mesh, PartitionSpec

    install_neuronx_cc_hook()
    partition_name = nc.partition_id_tensor.name if nc.partition_id_tensor else None
    in_names, out_names, out_avals, zero_outs = [], [], [], []
    for alloc in nc.m.functions[0].allocations:
        if not isinstance(alloc, mb.MemoryLocationSet):
            continue
        name = alloc.memorylocations[0].name
        if alloc.kind == "ExternalInput":
            if name != partition_name:
                in_names.append(name)
        elif alloc.kind == "ExternalOutput":
            out_names.append(name)
            shape = tuple(alloc.tensor_shape)
            dtype = mb.dt.np(alloc.dtype)
            out_avals.append(jax.core.ShapedArray(shape, dtype))
            zero_outs.append(np.zeros(shape, dtype))
    n_params = len(in_names)
    all_in_names = in_names + out_names
    if partition_name is not None:
        all_in_names.append(partition_name)

    def _body(*args):
        operands = list(args)
        if partition_name is not None:
            operands.append(partition_id_tensor())
        outs = _bass_exec_p.bind(
            *operands,
            out_avals=tuple(out_avals),
            in_names=tuple(all_in_names),
            out_names=tuple(out_names),
            lowering_input_output_aliases=(),
            sim_require_finite=True,
            sim_require_nnan=True,
            nc=nc,
        )
        return tuple(outs)

    devices = jax.devices()[:n_cores]
    mesh = Mesh(np.asarray(devices), ("core",))
    nspec = n_params + len(out_avals)
    fn = jax.jit(
        shard_map(
            _body,
            mesh=mesh,
            in_specs=(PartitionSpec("core"),) * nspec,
            out_specs=(PartitionSpec("core"),) * len(out_names),
            check_rep=False,
        ),
        keep_unused=True,
    )
    concat_in = [
        np.concatenate([np.asarray(m[name]) for m in in_maps], axis=0)
        for name in in_names
    ]
    concat_zero = [
        np.zeros((n_cores * z.shape[0], *z.shape[1:]), z.dtype) for z in zero_outs
    ]
    dev_args = [jax.device_put(a) for a in concat_in + concat_zero]
    outs = fn(*dev_args)
    jax.block_until_ready(outs)

    exec_ns = None
    if bench_iters > 0:
        t0 = time.perf_counter()
        for _ in range(bench_iters):
            outs_b = fn(*dev_args)
        jax.block_until_ready(outs_b)
        exec_ns = (time.perf_counter() - t0) / bench_iters * 1e9

    results = []
    for c in range(n_cores):
        res = {}
        for i, name in enumerate(out_names):
            full = np.asarray(outs[i])
            per = full.shape[0] // n_cores
            res[name] = full[c * per : (c + 1) * per]
        results.append(res)
    return results, exec_ns


def kernel(**inputs):
    import os

    x, wk16, wv16, ident, identf, g, b = _host_inputs(inputs)
    n_el_total = x.shape[0]
    assert n_el_total % (N_CORES * SUP * CH) == 0
    bc = n_el_total // N_CORES
    n_super = bc // (SUP * CH)

    ln_trivial = bool(np.allclose(g, 1.0) and np.allclose(b, 0.0))
    nc = build_nc(n_super, ln_trivial=ln_trivial)
    in_maps = []
    for i in range(N_CORES):
        in_maps.append(
            {
                "x": x[i * bc : (i + 1) * bc],
                "wk": wk16,
                "wv": wv16,
                "ident": ident,
                "identf": identf,
                "ln_g": g,
                "ln_b": b,
            }
        )
    bench = int(os.environ.get("KERNEL_BENCH", "0"))
    global LAST_EXEC_TIME_NS
    if bench > 0:
        results, exec_ns = _run_and_bench(nc, in_maps, N_CORES, bench_iters=bench)
        LAST_EXEC_TIME_NS = exec_ns
        y = np.concatenate([results[i]["y"] for i in range(N_CORES)], axis=0)
    else:
        rr = run_bass_kernel_spmd(nc, in_maps, list(range(N_CORES)))
        LAST_EXEC_TIME_NS = rr.exec_time_ns
        y = np.concatenate([rr.results[i]["y"] for i in range(N_CORES)], axis=0)
    return y.reshape(np.asarray(inputs["x"]).shape)


LAST_EXEC_TIME_NS = None
